# revision 1
# baseline (speedup 1.0000x reference)
"""GCN-Tox21 GNN message-passing kernel for 8 Trainium2 NeuronCores.

Strategy (graph/edge parallelism):
  - Sort edges by destination node on the host; core k owns the destination
    node range [k*NPC, (k+1)*NPC) and all edges pointing into it.
  - Node features h live replicated in each core's DRAM (bf16), laid out
    partition-major per rank: global row id for node n (core k, local r) is
    k*NPC + (r%128)*W + r//128, so a whole layer's h is written with ONE
    [128, W*F] DMA and AllGather'd as one block. Host-side gather indices
    are permuted to match.
  - Per-edge gathers of h[src] use dma_gather(transpose=True), which lands
    features feature-major in SBUF, ready as matmul moving operands.
  - The per-edge 2-layer MLP runs on the tensor engine in bf16 with fp32
    PSUM accumulation. b1 is folded in via a constant-ones row appended to
    the e^T operand; b2 is added with a broadcast tile on the vector engine.
  - Segment-sum to destination nodes is a matmul with host-built 0/1 one-hot
    tiles (edges sorted by dst => each 128-node window's edges are
    contiguous; PSUM accumulates across the window's edge subtiles).
  - Mean + eval-mode BN fold into h = relu((seg_sum + cnt'*cb) * invcnt),
    cb = bn_b - bn_m*A, A = g/sqrt(rv+eps); w2/b2 pre-scaled by A. The
    rank-1 cnt'*cb term is one K=1 matmul per window.
  - After each conv layer an AllGather rebuilds the replicated h. h0 is
    likewise computed own-stripe only and AllGather'd.
  - Mean-pool + FC + sigmoid: core k handles graphs [k*GPC, (k+1)*GPC)
    (batch is sorted, so their nodes are contiguous; dma_gather fetches
    them node-major for the pooling matmul).
  - DMAs are batched aggressively (each DMA instruction costs ~625ns on the
    shared HWDGE path): one S load per 512-edge group, edge features in
    2048-wide chunks, one h write per layer.
"""

import numpy as np
import ml_dtypes

import concourse.bacc as bacc
import concourse.tile as tile
from concourse import mybir, bass_utils
from concourse.masks import make_identity

BF16 = mybir.dt.bfloat16
F32 = mybir.dt.float32
I16 = mybir.dt.int16
RELU = mybir.ActivationFunctionType.Relu

N_CORES = 8
BN_EPS = 1e-5
G_REAL = 512
F_NODE, F_EDGE, H, EH = 32, 8, 256, 16
OUT_DIMS = (256, 256, 128)
EG = 512  # edges per gather batch


def _bf(a):
    return np.ascontiguousarray(a.astype(ml_dtypes.bfloat16))


def _f32(a):
    return np.ascontiguousarray(a.astype(np.float32))


def _wrap_idx(idx):
    """int16 index layout for dma_gather: index i at [i % 16, i // 16],
    replicated across the 8 partition groups."""
    assert len(idx) % 16 == 0
    w = idx.astype(np.int16).reshape(-1, 16).T
    return np.ascontiguousarray(np.tile(w, (8, 1)))


class Plan:
    """Host-side preprocessing: sharding layout + per-core input tensors."""

    def __init__(self, inputs, G):
        x = np.asarray(inputs["x"]).astype(np.float32)
        N = x.shape[0]
        self.N, self.G = N, G
        self.N_pad = ((N + N_CORES * 128 - 1) // (N_CORES * 128)) * (N_CORES * 128)
        self.NPC = self.N_pad // N_CORES
        self.W = self.NPC // 128
        assert G % N_CORES == 0
        self.GPC = G // N_CORES

        edge_index = np.asarray(inputs["edge_index"]).astype(np.int64)
        src, dst = edge_index[0].astype(np.int32), edge_index[1].astype(np.int32)
        batch = np.asarray(inputs["batch"]).astype(np.int32)
        edge_attr = np.asarray(inputs["edge_attr"]).astype(np.float32)

        order = np.argsort(dst, kind="stable")
        s_dst, s_src = dst[order], src[order]
        s_ea = edge_attr[order]

        bounds = np.searchsorted(s_dst, np.arange(0, self.N_pad + 1, 128), "left")
        cnt_all = bounds[1:] - bounds[:-1]
        t_all = np.maximum(1, -(-cnt_all // 128))
        NW = len(t_all)
        # balanced window->core assignment: snake-deal windows sorted by
        # descending tile count, so every core gets a near-identical
        # per-slot tile-count profile (the SPMD program uses the max).
        order_w = np.argsort(-t_all, kind="stable")
        slots = [[] for _ in range(N_CORES)]
        for pos, win in enumerate(order_w):
            rnd, r = divmod(pos, N_CORES)
            k = r if rnd % 2 == 0 else N_CORES - 1 - r
            slots[k].append(int(win))
        self.slots = slots
        owner = np.zeros(NW, np.int64)
        slot = np.zeros(NW, np.int64)
        for k, lst in enumerate(slots):
            for j, winid in enumerate(lst):
                owner[winid] = k
                slot[winid] = j
        T_w = np.array([max(t_all[slots[k][j]] for k in range(N_CORES))
                        for j in range(self.W)])
        while T_w.sum() % (EG // 128) != 0:
            T_w[-1] += 1
        self.T_w = [int(t) for t in T_w]
        self.T_tot = int(T_w.sum())
        self.ET = self.T_tot * 128

        # flush-major row permutation: node n -> DRAM row of zs_full/h_full.
        # AllGathers run per 4-window flush; flush m of all ranks lands in
        # the contiguous block [m*4096, (m+1)*4096).
        def rowperm(n):
            win = n // 128
            p = n % 128
            m = slot[win] // 4
            return (m * (N_CORES * 512) + owner[win] * 512
                    + (slot[win] % 4) * 128 + p)

        cnt = np.bincount(dst, minlength=self.N_pad).astype(np.float32)
        invc_full = 1.0 / np.maximum(cnt, 1.0)
        cntp_full = np.maximum(cnt, 1.0)
        gcnt = np.bincount(batch, minlength=G).astype(np.float32)
        ginv_full = 1.0 / np.maximum(gcnt, 1.0)

        lo_k = [int(np.searchsorted(batch, k * self.GPC, "left")) for k in range(N_CORES)]
        hi_k = [int(np.searchsorted(batch, (k + 1) * self.GPC, "left")) for k in range(N_CORES)]
        self.TP = max(1, max(-(-(h - l) // 128) for l, h in zip(lo_k, hi_k)))
        self.NPOOL = self.TP * 128

        self.per_core = []
        for k in range(N_CORES):
            d = {}
            gi_src = np.zeros(self.ET, np.int32)
            ea_pad = np.zeros((self.ET, F_EDGE), np.float32)
            S = np.zeros((128, self.ET), np.float32)
            pos = 0
            for w in range(self.W):
                base = slots[k][w] * 128
                lo = np.searchsorted(s_dst, base, "left")
                hi = np.searchsorted(s_dst, base + 128, "left")
                n = hi - lo
                sl = slice(pos, pos + n)
                gi_src[sl] = s_src[lo:hi]
                ea_pad[sl] = s_ea[lo:hi]
                loc = (s_dst[lo:hi] - base).astype(np.int64)
                e_ids = np.arange(pos, pos + n)
                S[e_ids % 128, (e_ids // 128) * 128 + loc] = 1.0
                pos += self.T_w[w] * 128
            assert pos == self.ET

            d["gidx_src"] = _wrap_idx(rowperm(gi_src))
            d["S"] = _bf(S)
            # transposed one-hot: S_T[n, t*128+p] = S[p, t*128+n]
            ST = np.ascontiguousarray(
                S.reshape(128, self.T_tot, 128).transpose(2, 1, 0)
                .reshape(128, self.ET))
            d["S_T"] = _bf(ST)
            eaT = np.concatenate([ea_pad.T, np.ones((1, self.ET), np.float32)], 0)
            d["eaT"] = _bf(eaT)
            win_ids = np.array(slots[k])
            nidx = win_ids[None, :] * 128 + np.arange(128)[:, None]  # [128, W]
            d["invc"] = _f32(invc_full[nidx])
            d["cntrow"] = _bf(cntp_full[nidx.T.reshape(-1)].reshape(1, self.NPC))
            lo, hi = lo_k[k], hi_k[k]
            pidx = np.zeros(self.NPOOL, np.int32)
            pidx[: hi - lo] = np.arange(lo, hi)
            d["pool_idx"] = _wrap_idx(rowperm(pidx))
            S2 = np.zeros((128, self.TP * self.GPC), np.float32)
            pb = batch[lo:hi] - k * self.GPC
            e_ids = np.arange(hi - lo)
            S2[e_ids % 128, (e_ids // 128) * self.GPC + pb] = 1.0
            d["S2"] = _bf(S2)
            d["ginv"] = _f32(ginv_full[k * self.GPC:(k + 1) * self.GPC].reshape(self.GPC, 1))
            self.per_core.append(d)

        sh = {}
        x_pad = np.zeros((self.N_pad, F_NODE), np.float32)
        x_pad[:N] = x
        xT_full = np.concatenate([x_pad.T, np.ones((1, self.N_pad), np.float32)], 0)
        for k in range(N_CORES):
            win_ids = np.array(slots[k])
            cols = (win_ids[:, None] * 128 + np.arange(128)[None, :]).reshape(-1)
            self.per_core[k]["xT_own"] = _bf(xT_full[:, cols])
        ne_w, ne_b = _f32(inputs["ne_w"]), _f32(inputs["ne_b"])
        sh["ne_wT"] = _bf(np.concatenate([ne_w.T, ne_b[None, :]], 0))
        ee_w, ee_b = _f32(inputs["ee_w"]), _f32(inputs["ee_b"])
        sh["ee_wT"] = _bf(np.concatenate([ee_w.T, ee_b[None, :]], 0))

        in_dim = H
        self.layer_dims = []
        for i, out_dim in enumerate(OUT_DIMS):
            w1 = _f32(inputs[f"c{i}_w1"]); b1 = _f32(inputs[f"c{i}_b1"])
            w2 = _f32(inputs[f"c{i}_w2"]); b2 = _f32(inputs[f"c{i}_b2"])
            g = _f32(inputs[f"bn{i}_g"]); bb = _f32(inputs[f"bn{i}_b"])
            rm = _f32(inputs[f"bn{i}_m"]); rv = _f32(inputs[f"bn{i}_v"])
            A = g / np.sqrt(rv + BN_EPS)
            F_mid = 2 * out_dim
            # K-order: [h_dst(in), h_src(in), e(EH), ones]
            sh[f"w1T_{i}"] = _bf(np.concatenate([w1.T, b1[None, :]], 0))
            sh[f"w2T_{i}"] = _bf((w2 * A[:, None]).T)
            sh[f"cbrow_{i}"] = _bf((b2 * A + bb - rm * A)[None, :])
            self.layer_dims.append((in_dim, F_mid, out_dim))
            in_dim = out_dim

        fc_w, fc_b = _f32(inputs["fc_w"]), _f32(inputs["fc_b"])
        self.F_FC = fc_w.shape[0]
        sh["fc_wT"] = _bf(fc_w.T)
        sh["fcb_bc"] = _f32(np.tile(fc_b[None, :], (self.GPC, 1)))
        self.shared = sh

    def in_maps(self):
        return [{**self.shared, **self.per_core[k]} for k in range(N_CORES)]

def build_program(plan: Plan, n_cores=N_CORES, debug_no_collective=False,
                  debug_stage=9, repeats=1, skip_gather=False, skip_compute=False):
    nc = bacc.Bacc("TRN2", target_bir_lowering=False, debug=False,
                   num_devices=n_cores)

    ET, T_w, W, NPC, TP, GPC = plan.ET, plan.T_w, plan.W, plan.NPC, plan.TP, plan.GPC
    N_pad, NPOOL, F_FC = plan.N_pad, plan.NPOOL, plan.F_FC

    sample = plan.in_maps()[0]
    t_in = {name: nc.dram_tensor(name, list(arr.shape),
                                 mybir.dt.from_np(arr.dtype), kind="ExternalInput")
            for name, arr in sample.items()}
    out_part = nc.dram_tensor("out_part", [GPC, F_FC], F32, kind="ExternalOutput")

    n_batches = ET // EG

    # subtile -> window mapping (static)
    sub_window, sub_first, sub_last = [], [], []
    for w in range(W):
        for t in range(T_w[w]):
            sub_window.append(w)
            sub_first.append(t == 0)
            sub_last.append(t == T_w[w] - 1)

    with tile.TileContext(nc) as tc:
        with (
            tc.tile_pool(name="const", bufs=1) as cpool,
            tc.tile_pool(name="sbuf", bufs=2) as spool,
            tc.tile_pool(name="gath", bufs=3) as gpool,
            tc.tile_pool(name="m1sb", bufs=8) as m1pool,
            tc.tile_pool(name="psum", bufs=2, space="PSUM") as ppool,
            tc.tile_pool(name="dram", bufs=1, space="DRAM") as dpool,
        ):
            def _body():
                # ---------- resident constants ----------
                def load_const(name, dtype=None, tag=None):
                    arr = sample[name]
                    t = cpool.tile(list(arr.shape), dtype or mybir.dt.from_np(arr.dtype),
                                   tag=tag or name)
                    nc.sync.dma_start(out=t[:], in_=t_in[name][:])
                    return t

                # stage-A-critical loads first so PE starts immediately;
                # bulk tensors (S_T etc.) stream in behind them.
                xo_all = spool.tile([F_NODE + 1, NPC], BF16, tag="xoall",
                                    bufs=1)
                nc.sync.dma_start(out=xo_all[:], in_=t_in["xT_own"][:])
                ne_wT_t = load_const("ne_wT")
                ident = cpool.tile([128, 128], BF16, tag="ident")
                make_identity(nc, ident[:])

                w1T_t, w2T_t, cbrow_t = [], [], []

                def load_layer_weights(i):
                    F_in, F_mid, F_out = plan.layer_dims[i]
                    KC2 = 2 * F_in // 128
                    chunks = []
                    for kc in range(KC2):
                        t = cpool.tile([128, F_mid], BF16, tag=f"w1T_{i}_{kc}")
                        nc.sync.dma_start(out=t[:],
                                          in_=t_in[f"w1T_{i}"][kc * 128:(kc + 1) * 128, :])
                        chunks.append(t)
                    te = cpool.tile([EH + 1, F_mid], BF16, tag=f"w1Te_{i}")
                    nc.sync.dma_start(out=te[:],
                                      in_=t_in[f"w1T_{i}"][2 * F_in:2 * F_in + EH + 1, :])
                    w1T_t.append((chunks, te))
                    wc = []
                    for km in range(F_mid // 128):
                        t = cpool.tile([128, F_out], BF16, tag=f"w2T_{i}_{km}")
                        nc.sync.dma_start(out=t[:],
                                          in_=t_in[f"w2T_{i}"][km * 128:(km + 1) * 128, :])
                        wc.append(t)
                    w2T_t.append(wc)
                    cbrow_t.append(load_const(f"cbrow_{i}"))

                load_layer_weights(0)
                hT_t = [cpool.tile([128, W, F_in_ // 128, 128], BF16, tag="hT",
                                   name=f"hT{i}", bufs=2)
                        for i, (F_in_, _, _) in enumerate(plan.layer_dims)]
                gidx_src = load_const("gidx_src")
                invc_t = load_const("invc")
                cntrow_t = load_const("cntrow")
                ee_wT_t = load_const("ee_wT")

                # ---------- DRAM buffers ----------
                zs_own = [dpool.tile([NPC, d[1]], BF16, tag=f"zsown{i}",
                                     name=f"zs_own{i}")
                          for i, d in enumerate(plan.layer_dims)]
                zs_full = [dpool.tile([n_cores * NPC, d[1]], BF16,
                                      tag=f"zsfull{i}", name=f"zs_full{i}")
                           for i, d in enumerate(plan.layer_dims)]
                F_last = plan.layer_dims[-1][2]
                h_own3 = dpool.tile([NPC, F_last], BF16, tag="hown3",
                                    name="h_own3")
                h_full3 = dpool.tile([n_cores * NPC, F_last], BF16,
                                     tag="hfull3", name="h_full3")
                eT_dram = dpool.tile([EH + 1, ET], BF16, tag="eT")

                def allgather(src, dst, m):
                    # gather 4-window flush m: own rows [m*512, (m+1)*512)
                    # -> contiguous block [m*4096, (m+1)*4096) of dst
                    if debug_no_collective:
                        cp = spool.tile([128, 128], BF16, tag="dbgcp")
                        nc.sync.dma_start(out=cp[:],
                                          in_=src[m * 512:m * 512 + 128, 0:128])
                        nc.sync.dma_start(
                            out=dst[m * n_cores * 512:m * n_cores * 512 + 128,
                                    0:128], in_=cp[:])
                    else:
                        nc.gpsimd.collective_compute(
                            "AllGather", mybir.AluOpType.bypass,
                            ins=[src[m * 512:(m + 1) * 512, :].opt()],
                            outs=[dst[m * n_cores * 512:
                                      (m + 1) * n_cores * 512, :].opt()],
                            replica_groups=[list(range(n_cores))])

                # Per-window Zs^{li} = h^{li}_win @ W1src^{li}.T, rolled out
                # to DRAM in 4-window chunks; AllGather'd at stage end.
                zroll_state = {}

                def zs_window(li, w):
                    F_in_l, F_mid_l, _ = plan.layer_dims[li]
                    KCl = F_in_l // 128
                    zps = ppool.tile([128, F_mid_l], F32, tag="m1", bufs=4)
                    for kc in range(KCl):
                        nc.tensor.matmul(out=zps[:], lhsT=hT_t[li][:, w, kc, :],
                                         rhs=w1T_t[li][0][KCl + kc][:],
                                         start=(kc == 0), stop=(kc == KCl - 1))
                    zsb = spool.tile([128, F_mid_l], BF16, tag="zroll", bufs=3)
                    nc.scalar.activation(
                        out=zsb[:], in_=zps[:],
                        func=mybir.ActivationFunctionType.Copy)
                    nc.sync.dma_start(
                        out=zs_own[li][w * 128:(w + 1) * 128, :], in_=zsb[:])
                    if w % 4 == 3:
                        allgather(zs_own[li], zs_full[li], w // 4)

                # ---------- stage A: h0 = relu(x @ ne_w.T + ne_b), own stripe ----------
                for w in range(W):
                    ps = ppool.tile([128, H], F32, tag="m2")
                    nc.tensor.matmul(out=ps[:], lhsT=xo_all[:, w * 128:(w + 1) * 128],
                                     rhs=ne_wT_t[:], start=True, stop=True)
                    hsb = spool.tile([128, H], BF16, tag="hsb", bufs=3)
                    nc.scalar.activation(out=hsb[:], in_=ps[:], func=RELU)
                    for kc in range(H // 128):
                        tp = ppool.tile([128, 128], BF16, tag="m2")
                        nc.tensor.transpose(out=tp[:],
                                            in_=hsb[:, kc * 128:(kc + 1) * 128],
                                            identity=ident[:])
                        nc.vector.tensor_copy(out=hT_t[0][:, w, kc, :], in_=tp[:])
                    zs_window(0, w)

                # deferred bulk constants (overlap with stage A2 / layer 0)
                ST_t = load_const("S_T")
                load_layer_weights(1)
                load_layer_weights(2)
                pool_idx = load_const("pool_idx")
                S2_t = load_const("S2")
                ginv_t = load_const("ginv")
                fc_wT_t = load_const("fc_wT")
                fcb_t = load_const("fcb_bc")

                # ---------- stage A2: e^T (+ones row) -> DRAM [EH+1, ET] ----------
                if debug_stage < 2:
                    return
                EB = 2048
                for b0 in range((ET + EB - 1) // EB):
                    ew = min(EB, ET - b0 * EB)
                    ea_t = spool.tile([F_EDGE + 1, EB], BF16, tag="eaT")
                    nc.sync.dma_start(out=ea_t[:, 0:ew],
                                      in_=t_in["eaT"][:, b0 * EB:b0 * EB + ew])
                    et_sb = spool.tile([EH + 1, EB], BF16, tag="etsb")
                    nc.vector.memset(et_sb[:], 1.0)
                    for c in range(ew // 512):
                        ps = ppool.tile([EH, 512], F32, tag="m2")
                        nc.tensor.matmul(out=ps[:], lhsT=ee_wT_t[:],
                                         rhs=ea_t[:, c * 512:(c + 1) * 512],
                                         start=True, stop=True)
                        nc.scalar.activation(out=et_sb[0:EH, c * 512:(c + 1) * 512],
                                             in_=ps[:], func=RELU)
                    nc.sync.dma_start(out=eT_dram[:, b0 * EB:b0 * EB + ew],
                                      in_=et_sb[:, 0:ew])

                # ---------- conv layers ----------
                if debug_stage < 3:
                    return
                for li, (F_in, F_mid, F_out) in enumerate(plan.layer_dims):
                    KC = F_in // 128
                    MC = F_mid // 128
                    # hoisted per-window Q^T = h_win @ W1dst.T
                    qall = spool.tile([128, W, F_mid], BF16, tag="qall", bufs=1)
                    for w in range(W):
                        qtp = ppool.tile([128, F_mid], F32, tag="m1", bufs=4)
                        for kc in range(KC):
                            nc.tensor.matmul(
                                out=qtp[:], lhsT=hT_t[li][:, w, kc, :],
                                rhs=w1T_t[li][0][kc][:],
                                start=(kc == 0), stop=(kc == KC - 1))
                        nc.vector.tensor_copy(out=qall[:, w, :], in_=qtp[:])

                    hlbuf = None
                    node_ps = None

                    def part_b(t_glob, w, m1s, stt):
                        nonlocal hlbuf, node_ps
                        s = t_glob % 4
                        if sub_first[t_glob]:
                            node_ps = ppool.tile([128, MC, 128], F32, tag="node")
                        for fc in range(MC):
                            nc.tensor.matmul(
                                out=node_ps[:, fc, :],
                                lhsT=m1s[:, fc * 128:(fc + 1) * 128],
                                rhs=stt[:, s * 128:(s + 1) * 128],
                                start=sub_first[t_glob], stop=sub_last[t_glob],
                                skip_group_check=True)
                        if not sub_last[t_glob]:
                            return
                        ntsb = spool.tile([128, MC, 128], BF16, tag="nodeT",
                                          bufs=2)
                        nc.vector.tensor_copy(out=ntsb[:], in_=node_ps[:])
                        out2 = ppool.tile([128, F_out], F32, tag="m2")
                        nc.tensor.matmul(
                            out=out2[:],
                            lhsT=cntrow_t[0:1, w * 128:(w + 1) * 128],
                            rhs=cbrow_t[li][:], start=True, stop=False,
                            skip_group_check=True)
                        for fm in range(MC):
                            nc.tensor.matmul(
                                out=out2[:], lhsT=ntsb[:, fm, :],
                                rhs=w2T_t[li][fm][:],
                                start=False, stop=(fm == MC - 1),
                                skip_group_check=True)
                        if li < 2:
                            hsb = spool.tile([128, F_out], BF16, tag="hsb",
                                             bufs=3)
                            nc.scalar.activation(out=hsb[:], in_=out2[:],
                                                 func=RELU,
                                                 scale=invc_t[:, w:w + 1])
                            for kc in range(F_out // 128):
                                tp = ppool.tile([128, 128], BF16, tag="m2")
                                nc.tensor.transpose(
                                    out=tp[:],
                                    in_=hsb[:, kc * 128:(kc + 1) * 128],
                                    identity=ident[:])
                                nc.vector.tensor_copy(
                                    out=hT_t[li + 1][:, w, kc, :], in_=tp[:])
                            zs_window(li + 1, w)
                        else:
                            hsb = spool.tile([128, F_out], BF16, tag="hsb",
                                             bufs=3)
                            nc.scalar.activation(
                                out=hsb[:], in_=out2[:], func=RELU,
                                scale=invc_t[:, w:w + 1])
                            nc.sync.dma_start(
                                out=h_own3[w * 128:(w + 1) * 128, :],
                                in_=hsb[:])
                            if w % 4 == 3:
                                allgather(h_own3, h_full3, w // 4)

                    pend = []
                    zgs = None
                    for g in range(n_batches):
                        if g % 2 == 0:
                            ng = min(1024, ET - g * 512)
                            zgs = gpool.tile([128, 8, F_mid], BF16, tag="zgs")
                            if not skip_gather:
                                nc.gpsimd.dma_gather(
                                    zgs[:, 0:ng // 128, :], zs_full[li][:, :],
                                    gidx_src[:, g * 32:g * 32 + ng // 16],
                                    ng, ng, F_mid, transpose=False)
                        if g % 4 == 0:
                            ew = min(2048, ET - g * 512)
                            et_t = spool.tile([EH + 1, 2048], BF16,
                                              tag="et_in", bufs=2)
                            nc.sync.dma_start(
                                out=et_t[:, 0:ew],
                                in_=eT_dram[:, g * 512:g * 512 + ew])
                        stt = spool.tile([128, 512], BF16, tag="s_in", bufs=3)
                        nc.sync.dma_start(
                            out=stt[:],
                            in_=t_in["S"][:, g * 512:(g + 1) * 512])
                        if skip_compute:
                            continue
                        new_pend = []
                        for s in range(4):
                            t_glob = g * 4 + s
                            w = sub_window[t_glob]
                            # Zs-add engine routing: PE identity-matmul for
                            # 2/16 of subtiles, Pool for 2/16, DVE for the rest
                            r16 = t_glob % 16
                            pe_add = r16 in (0, 5, 10)
                            pool_add = False
                            m1p = ppool.tile([128, F_mid], F32, tag="m1",
                                             bufs=4)
                            nc.tensor.matmul(
                                out=m1p[:],
                                lhsT=ST_t[:, t_glob * 128:(t_glob + 1) * 128],
                                rhs=qall[:, w, :],
                                start=True, stop=False, skip_group_check=True)
                            e4 = (g % 4) * 512 + s * 128
                            nc.tensor.matmul(
                                out=m1p[:], lhsT=et_t[:, e4:e4 + 128],
                                rhs=w1T_t[li][1][:],
                                start=False, stop=not pe_add,
                                skip_group_check=True)
                            m1s = m1pool.tile([128, F_mid], BF16, tag="m1sb")
                            if pe_add:
                                nc.tensor.matmul(
                                    out=m1p[:], lhsT=ident[:],
                                    rhs=zgs[:, (g % 2) * 4 + s, :],
                                    start=False, stop=True,
                                    skip_group_check=True)
                                nc.scalar.activation(out=m1s[:], in_=m1p[:],
                                                     func=RELU)
                            else:
                                tmp = spool.tile([128, F_mid], BF16,
                                                 tag="m1add", bufs=4)
                                eng = nc.gpsimd if pool_add else nc.vector
                                eng.tensor_tensor(out=tmp[:], in0=m1p[:],
                                                  in1=zgs[:, (g % 2) * 4 + s, :],
                                                  op=mybir.AluOpType.add)
                                if li == 2 and t_glob % 2 == 1:
                                    nc.vector.tensor_scalar_max(
                                        out=m1s[:], in0=tmp[:], scalar1=0.0)
                                else:
                                    nc.scalar.activation(out=m1s[:], in_=tmp[:],
                                                         func=RELU)
                            new_pend.append((t_glob, w, m1s, stt))
                        for item in pend:
                            part_b(*item)
                        pend = new_pend
                    for item in pend:
                        part_b(*item)

                # ---------- pooling + FC + sigmoid ----------
                if debug_stage < 5:
                    return
                hp = spool.tile([128, TP, F_last], BF16, tag="hp", bufs=1)
                # gather in <=512-index chunks (larger single gathers crash)
                for p0 in range(0, TP, 4):
                    pn = min(4, TP - p0)
                    nc.gpsimd.dma_gather(
                        hp[:, p0:p0 + pn, :],
                        h_full3[:, :],
                        pool_idx[:, p0 * 8:(p0 + pn) * 8],
                        pn * 128, pn * 128, F_last, transpose=False)
                if debug_stage < 6:
                    return
                pool_ps = ppool.tile([GPC, F_last], F32, tag="m1", bufs=4)
                for t in range(TP):
                    nc.tensor.matmul(out=pool_ps[:],
                                     lhsT=S2_t[:, t * GPC:(t + 1) * GPC],
                                     rhs=hp[:, t, :], start=(t == 0), stop=(t == TP - 1))
                pooled_sb = spool.tile([GPC, F_last], BF16, tag="pooled")
                nc.scalar.activation(out=pooled_sb[:], in_=pool_ps[:],
                                     func=mybir.ActivationFunctionType.Copy,
                                     scale=ginv_t[:])
                if debug_stage < 7:
                    return
                ptr_ps = ppool.tile([F_last, GPC], BF16, tag="m2")
                nc.tensor.transpose(out=ptr_ps[:], in_=pooled_sb[:],
                                    identity=ident[0:GPC, 0:GPC])
                ptr_sb = spool.tile([F_last, GPC], BF16, tag="ptrsb")
                nc.vector.tensor_copy(out=ptr_sb[:], in_=ptr_ps[:])
                if debug_stage < 8:
                    return
                fc_ps = ppool.tile([GPC, F_FC], F32, tag="node")
                nc.tensor.matmul(out=fc_ps[:], lhsT=ptr_sb[:], rhs=fc_wT_t[:],
                                 start=True, stop=True)
                logit = spool.tile([GPC, F_FC], F32, tag="logit")
                nc.vector.tensor_tensor(out=logit[:], in0=fc_ps[:], in1=fcb_t[:],
                                        op=mybir.AluOpType.add)
                # Sigmoid's activation-table load (sigmoid_and_friends) crashes
                # this runtime; the host applies the exact fp32 sigmoid instead.
                nc.sync.dma_start(out=out_part[:], in_=logit[:])

            for _r in range(repeats):
                _body()

    nc.compile()
    return nc

_CACHE = {}


def run(inputs, G=G_REAL):
    plan = Plan(inputs, G)
    key = (plan.N, plan.G, plan.TP, tuple(plan.T_w))
    if key not in _CACHE:
        _CACHE[key] = build_program(plan)
    nc = _CACHE[key]
    res = bass_utils.run_bass_kernel_spmd(nc, plan.in_maps(),
                                          core_ids=list(range(N_CORES)))
    logits = np.concatenate([res.results[k]["out_part"] for k in range(N_CORES)], 0)
    out = 1.0 / (1.0 + np.exp(-logits.astype(np.float64)))
    return np.ascontiguousarray(out.astype(np.float32))


def kernel(**inputs) -> np.ndarray:
    return run(inputs, G=G_REAL)

